# revision 1
# baseline (speedup 1.0000x reference)
import sys

sys.path.insert(0, "/opt/trn_rl_repo")

import numpy as np

import concourse.bass as bass
import concourse.bacc as bacc
import concourse.mybir as mybir
from concourse.tile import TileContext
from concourse.bass_utils import run_bass_kernel_spmd

try:
    from ml_dtypes import bfloat16 as np_bf16
except ImportError:  # pragma: no cover
    import jax.numpy as _jnp

    np_bf16 = _jnp.bfloat16

P = 128          # partitions
BT = 512         # batch-tile (free dim) per matmul
G = 4            # batch groups packed into 128 partitions for the GRU
NCORES = 8
B, S, H, A = 131072, 256, 512, 32
BC = B // NCORES           # 16384 rows per core
MACRO = G * BT             # 2048 rows per GRU macro-tile
NM = BC // MACRO           # 8 macro-tiles per core
NG = BC // BT              # 32 MLP group-tiles per core

FP32 = mybir.dt.float32
BF16 = mybir.dt.bfloat16
AF = mybir.ActivationFunctionType
OP = mybir.AluOpType

_CACHE = {}


def _build(nsteps: int) -> bass.Bass:
    nc = bacc.Bacc("TRN2", target_bir_lowering=False, debug=False,
                   num_devices=NCORES)

    xd = nc.dram_tensor("xd", [P, 2, BC], BF16, kind="ExternalInput")
    w1d = nc.dram_tensor("w1d", [P, 2, H], BF16, kind="ExternalInput")
    w2d = nc.dram_tensor("w2d", [P, 4, H], BF16, kind="ExternalInput")
    wmd = nc.dram_tensor("wmd", [P, 4, A], BF16, kind="ExternalInput")
    b1d = nc.dram_tensor("b1d", [P, 4], FP32, kind="ExternalInput")
    b2d = nc.dram_tensor("b2d", [P, 4], FP32, kind="ExternalInput")
    bmd = nc.dram_tensor("bmd", [A, 1], FP32, kind="ExternalInput")
    # augmented input-gate weights: rows 0-7 kron(I4, w_ih_gate.T), row 8 bias
    lrid = nc.dram_tensor("lrid", [2 * G + 1, P], BF16, kind="ExternalInput")
    luid = nc.dram_tensor("luid", [2 * G + 1, P], BF16, kind="ExternalInput")
    lnid = nc.dram_tensor("lnid", [2 * G + 1, P], BF16, kind="ExternalInput")
    lrhd = nc.dram_tensor("lrhd", [P, P], BF16, kind="ExternalInput")
    luhd = nc.dram_tensor("luhd", [P, P], BF16, kind="ExternalInput")
    lnhd = nc.dram_tensor("lnhd", [P, P], BF16, kind="ExternalInput")
    lwd = nc.dram_tensor("lwd", [P, NM, P], BF16, kind="ExternalInput")
    eyed = nc.dram_tensor("eyed", [P, P], BF16, kind="ExternalInput")
    bnhd = nc.dram_tensor("bnhd", [P, 1], FP32, kind="ExternalInput")
    bwtd = nc.dram_tensor("bwtd", [P, nsteps], FP32, kind="ExternalInput")
    zod = nc.dram_tensor("zod", [2 * G + 1, nsteps + 1, NM, BT], BF16,
                         kind="ExternalInput")
    outd = nc.dram_tensor("outd", [2 * G, nsteps, NM, BT], BF16,
                          kind="ExternalOutput")

    with TileContext(nc) as tc:
        with (
            tc.tile_pool(name="const", bufs=1) as const,
            tc.tile_pool(name="state", bufs=1) as state,
            tc.tile_pool(name="xp", bufs=4) as xp,
            tc.tile_pool(name="actp", bufs=2) as actp,
            tc.tile_pool(name="gp", bufs=4) as gp,
            tc.tile_pool(name="psA", bufs=2, space="PSUM") as psA,
            tc.tile_pool(name="psB", bufs=2, space="PSUM") as psB,
            tc.tile_pool(name="psAcc", bufs=1, space="PSUM") as psAcc,
        ):
            # ---- MLP constants first so the first X DMA isn't queued
            # behind the GRU constants ----
            w1s = const.tile([P, 2, H], BF16)
            nc.sync.dma_start(w1s[:], w1d[:])
            b1s = const.tile([P, 4], FP32)
            nc.sync.dma_start(b1s[:], b1d[:])
            # L2/L3 constants on the Act-issued DMA queue: they transfer
            # in parallel with w1s/X on the SP queue during startup
            w2s = const.tile([P, 4, H], BF16)
            nc.scalar.dma_start(w2s[:], w2d[:])
            b2s = const.tile([P, 4], FP32)
            nc.scalar.dma_start(b2s[:], b2d[:])
            wms = const.tile([P, 4, A], BF16)
            nc.scalar.dma_start(wms[:], wmd[:])
            bms = const.tile([A, 1], FP32)
            nc.scalar.dma_start(bms[:], bmd[:])
            lris = const.tile([2 * G + 1, P], BF16)
            luis = const.tile([2 * G + 1, P], BF16)
            lnis = const.tile([2 * G + 1, P], BF16)
            lrhs = const.tile([P, P], BF16)
            luhs = const.tile([P, P], BF16)
            lnhs = const.tile([P, P], BF16)
            lws = const.tile([P, NM, P], BF16)
            eyes = const.tile([P, P], BF16)
            bnhs = const.tile([P, 1], FP32)
            bwts = const.tile([P, nsteps], FP32)

            def load_gru_consts():
                nc.sync.dma_start(lris[:], lrid[:])
                nc.sync.dma_start(luis[:], luid[:])
                nc.sync.dma_start(lnis[:], lnid[:])
                nc.sync.dma_start(lrhs[:], lrhd[:])
                nc.sync.dma_start(luhs[:], luhd[:])
                nc.sync.dma_start(lnhs[:], lnhd[:])
                nc.sync.dma_start(lws[:], lwd[:])
                nc.sync.dma_start(eyes[:], eyed[:])
                nc.sync.dma_start(bnhs[:], bnhd[:])
                nc.sync.dma_start(bwts[:], bwtd[:])
                nc.sync.dma_start(wpball[:], zod[:])

            # ---- persistent state ----
            # double-buffered by step parity: the Zn write must not WAR-wait
            # on the slow Pool-engine reads of the previous value
            Zb2 = []
            for par in range(2):
                row = []
                for m in range(NM):
                    z = state.tile([P, BT], BF16, tag=f"Z{par}_{m}",
                                   name=f"Zt{par}_{m}")
                    row.append(z)
                Zb2.append(row)
            Zb = Zb2[0]
            # all wp steps in one tile: rows 0-7 wp, row 8 ones (bias matmul);
            # slice [:, t, m, :] is macro m's wp after step t-1 (t=0 is init)
            wpball = state.tile([2 * G + 1, nsteps + 1, NM, BT], BF16,
                                tag="wpball", name="wpball")
            # wp accumulators: two PSUM banks; macro m lives at the
            # 32-aligned offset 32*(m%4) of bank m//4 (engine reads must
            # start on a 32-partition boundary)
            paccA = psAcc.tile([P, BT], FP32, tag="pA", name="paccA")
            paccB = psAcc.tile([P, BT], FP32, tag="pB", name="paccB")

            # ---- fused wavefront: MLP macros interleaved with GRU cells ----
            def relu_out(dst, src, bias, which):
                # src is PSUM: only Act and DVE may read it (GPSIMD cannot)
                if which == 0:
                    nc.scalar.activation(dst, src, AF.Relu, bias=bias)
                else:
                    nc.vector.tensor_scalar(dst, src, bias, 0.0, OP.add,
                                            OP.max)

            def mlp_pair(g0):
                # two batch groups in the free-dim halves of each PSUM tile:
                # one [128,1024] relu (same per-partition bias) serves both
                Xs = []
                for g in (g0, g0 + 1):
                    X = xp.tile([P, 2, BT], BF16, tag="X", name="X")
                    nc.sync.dma_start(X[:], xd[:, :, g * BT:(g + 1) * BT])
                    Xs.append(X)
                H1 = actp.tile([P, 4, 2, BT], BF16, tag="H1", name="H1")
                H2 = actp.tile([P, 4, 2, BT], BF16, tag="H2", name="H2")
                for f in range(4):
                    ps = psA.tile([P, 2, BT], FP32, tag="A", name="psL1")
                    for k in range(2):
                        for j in range(2):
                            nc.tensor.matmul(ps[:, j, :],
                                             w1s[:, k, f * P:(f + 1) * P],
                                             Xs[j][:, k, :], start=(k == 0),
                                             stop=(k == 1),
                                             skip_group_check=True)
                    relu_out(H1[:, f, :, :], ps[:], b1s[:, f:f + 1],
                             0 if f != 2 else 1)
                for f in range(4):
                    ps = psA.tile([P, 2, BT], FP32, tag="A", name="psL2")
                    for k in range(4):
                        for j in range(2):
                            nc.tensor.matmul(ps[:, j, :],
                                             w2s[:, k, f * P:(f + 1) * P],
                                             H1[:, k, j, :], start=(k == 0),
                                             stop=(k == 3),
                                             skip_group_check=True)
                    relu_out(H2[:, f, :, :], ps[:], b2s[:, f:f + 1],
                             0 if f != 2 else 1)
                ps3 = [psB.tile([A, BT], FP32, tag="B", name="psL3")[:]
                       for _ in range(2)]
                for k in range(4):
                    for j in range(2):
                        nc.tensor.matmul(ps3[j], wms[:, k, :],
                                         H2[:, k, j, :], start=(k == 0),
                                         stop=(k == 3), skip_group_check=True)
                for j in range(2):
                    g = g0 + j
                    m, q = g // G, g % G
                    if j == 0:
                        nc.vector.tensor_scalar_add(
                            Zb[m][q * A:(q + 1) * A, :], ps3[j],
                            bms[:, 0:1])
                    else:
                        nc.scalar.activation(Zb[m][q * A:(q + 1) * A, :],
                                             ps3[j], AF.Identity,
                                             bias=bms[:, 0:1])

            cellno = [0]

            def gru_cell(t, m, fast=False, c0=0, cw=BT):
                    par = cellno[0] % 2
                    cellno[0] += 1
                    wprev = wpball[:, t, m, c0:c0 + cw]
                    zsl = Zb2[t % 2][m][:, c0:c0 + cw]
                    znew = Zb2[(t + 1) % 2][m][:, c0:c0 + cw]
                    psRU = psA.tile([P, 2, cw], FP32, tag="A", name="psRU")
                    nc.tensor.matmul(psRU[:, 0, :], lrhs[:], zsl,
                                     start=True, stop=False)
                    nc.tensor.matmul(psRU[:, 0, :], lris[:], wprev,
                                     start=False, stop=True)
                    RU = gp.tile([P, 2, cw], BF16, tag="RU", name="RU")
                    if fast:
                        # latency form: R usable after its own two matmuls
                        nc.scalar.activation(RU[:, 0, :], psRU[:, 0, :],
                                             AF.Sigmoid)
                    nc.tensor.matmul(psRU[:, 1, :], luhs[:], zsl,
                                     start=True, stop=False)
                    nc.tensor.matmul(psRU[:, 1, :], luis[:], wprev,
                                     start=False, stop=True)
                    pB = psB.tile([P, cw], FP32, tag="B", name="pB")
                    nc.tensor.matmul(pB[:], lnhs[:], zsl,
                                     start=True, stop=True,
                                     skip_group_check=True)
                    if fast:
                        nc.scalar.activation(RU[:, 1, :], psRU[:, 1, :],
                                             AF.Sigmoid)
                    else:
                        nc.scalar.activation(RU[:], psRU[:], AF.Sigmoid)
                    HN = gp.tile([P, cw], BF16, tag="HN", name="HN")
                    nc.vector.tensor_scalar_add(HN[:], pB[:], bnhs[:, 0:1])
                    T1 = gp.tile([P, cw], BF16, tag="T1", name="T1")
                    nc.vector.tensor_tensor(T1[:], RU[:, 0, :], HN[:],
                                            OP.mult)
                    # reuse the bank: fresh group = lni@wp (start zeroes,
                    # issued off the critical path) + T1 via identity matmul
                    nc.tensor.matmul(pB[:], lnis[:], wprev,
                                     start=True, stop=False,
                                     skip_group_check=True)
                    nc.tensor.matmul(pB[:], eyes[:], T1[:],
                                     start=False, stop=True,
                                     skip_group_check=True)
                    NT = gp.tile([P, cw], BF16, tag="NT", name="NT")
                    nc.scalar.activation(NT[:], pB[:], AF.Tanh)
                    if fast:
                        # Zn = U*Zb + (1-U)*NT: only two hops after tanh
                        P1 = gp.tile([P, cw], BF16, tag="D", name="P1")
                        nc.gpsimd.tensor_tensor(P1[:], RU[:, 1, :], zsl,
                                                OP.mult)
                        OMU = gp.tile([P, cw], BF16, tag="E", name="OMU")
                        nc.vector.tensor_scalar(OMU[:], RU[:, 1, :], -1.0,
                                                1.0, OP.mult, OP.add)
                        P2 = gp.tile([P, cw], BF16, tag="P2", name="P2")
                        nc.vector.tensor_tensor(P2[:], OMU[:], NT[:], OP.mult)
                        nc.vector.tensor_tensor(znew, P1[:], P2[:],
                                                OP.add)
                    else:
                        D = gp.tile([P, cw], BF16, tag="D", name="D")
                        nc.gpsimd.tensor_tensor(D[:], zsl, NT[:],
                                                OP.subtract)
                        E = gp.tile([P, cw], BF16, tag="E", name="E")
                        if par == 0:
                            nc.gpsimd.tensor_tensor(E[:], RU[:, 1, :], D[:],
                                                    OP.mult)
                        else:
                            nc.vector.tensor_tensor(E[:], RU[:, 1, :], D[:],
                                                    OP.mult)
                        nc.vector.tensor_tensor(znew, E[:], NT[:],
                                                OP.add)
                    pacc = paccA if m < G else paccB
                    off = 32 * (m % G)
                    nc.tensor.matmul(pacc[:, c0:c0 + cw], lws[:, m, :], znew,
                                     start=(t == 0 and m % G == 0),
                                     stop=(t == nsteps - 1 and
                                           m % G == G - 1 and
                                           c0 + cw == BT),
                                     skip_group_check=True)
                    if fast and par == 0:
                        # drain only: Act shares the wp update (measured best)
                        nc.scalar.activation(
                            wpball[0:2 * G, t + 1, m, c0:c0 + cw],
                            pacc[off:off + 2 * G, c0:c0 + cw],
                            AF.Identity,
                            bias=bwts[off:off + 2 * G, t:t + 1])
                    else:
                        # dense: keep Act's queue clear so relus free the
                        # PSUM rotation ring sooner (PE stall source)
                        nc.vector.tensor_scalar_add(
                            wpball[0:2 * G, t + 1, m, c0:c0 + cw],
                            pacc[off:off + 2 * G, c0:c0 + cw],
                            bwts[off:off + 2 * G, t:t + 1])

            # frontier wavefront: each slot runs one macro's MLP and emits up
            # to two GRU cells per older macro (one batch between the MLP
            # pairs, one after), so same-macro cells sit >= one MLP pair
            # apart in the in-order engine queues.
            next_t = [0] * NM
            dma_t = [0]

            def batch(mm_max, dcap, fast=False):
                for m in range(mm_max):
                    t = next_t[m]
                    if t < nsteps and t + m <= dcap:
                        gru_cell(t, m, fast=fast)
                        next_t[m] = t + 1
                # one step of slack so the store's sems are resolved by the
                # time it reaches the SP queue head (no HOL block of X loads)
                while (dma_t[0] < nsteps
                       and all(nt > dma_t[0] + 1 for nt in next_t)):
                    t0 = dma_t[0]
                    nc.sync.dma_start(outd[:, t0, :, :],
                                      wpball[0:2 * G, t0 + 1, :, :])
                    dma_t[0] = t0 + 1

            def flush_dmas(slack):
                while (dma_t[0] < nsteps
                       and all(nt > dma_t[0] + slack for nt in next_t)):
                    t0 = dma_t[0]
                    nc.sync.dma_start(outd[:, t0, :, :],
                                      wpball[0:2 * G, t0 + 1, :, :])
                    dma_t[0] = t0 + 1

            for s in range(NM):
                batch(s, 2 * s - 2)
                mlp_pair(G * s)
                if s == 0:
                    load_gru_consts()
                batch(s, 2 * s - 1)
                mlp_pair(G * s + 2)
            # greedy drain weave: always run the most-starved macro, spaced
            # >= SPACING cells from its previous step so the in-order engine
            # queues never stall on its chain; the last macro is no longer
            # throttled to one step per full batch sweep
            SPACING = 4
            lastpos = [-SPACING] * NM
            pos = 0
            while any(nt < nsteps for nt in next_t):
                cand = [m for m in range(NM)
                        if next_t[m] < nsteps
                        and pos - lastpos[m] >= SPACING]
                if not cand:
                    cand = [m for m in range(NM) if next_t[m] < nsteps]
                m = max(cand, key=lambda mm: nsteps - next_t[mm])
                gru_cell(next_t[m], m, fast=True)
                next_t[m] += 1
                lastpos[m] = pos
                pos += 1
                flush_dmas(1)
            while dma_t[0] < nsteps:
                t0 = dma_t[0]
                nc.sync.dma_start(outd[:, t0, :, :],
                                  wpball[0:2 * G, t0 + 1, :, :])
                dma_t[0] = t0 + 1
    nc.compile()
    return nc


LAST_RESULT = None


def _lwd(Ww):
    I4 = np.eye(G, dtype=np.float32)
    blk = np.kron(I4, Ww.T)                      # [128, 8]
    out = np.zeros((P, NM, P), np.float32)
    for m in range(NM):
        off = 32 * (m % G)
        out[:, m, off:off + 2 * G] = blk
    return out.astype(np_bf16)


def _zod(nsteps):
    z = np.zeros((2 * G + 1, nsteps + 1, NM, BT), np.float32)
    z[2 * G] = 1.0
    return z.astype(np_bf16)


def _prep_common(inputs, nsteps):
    W1 = np.asarray(inputs["W1"], np.float32)
    b1 = np.asarray(inputs["b1"], np.float32)
    W2 = np.asarray(inputs["W2"], np.float32)
    b2 = np.asarray(inputs["b2"], np.float32)
    Wm = np.asarray(inputs["Wm"], np.float32)
    bm = np.asarray(inputs["bm"], np.float32)
    w_ih = np.asarray(inputs["w_ih"], np.float32)
    w_hh = np.asarray(inputs["w_hh"], np.float32)
    b_ih = np.asarray(inputs["b_ih"], np.float32)
    b_hh = np.asarray(inputs["b_hh"], np.float32)
    Ww = np.asarray(inputs["Ww"], np.float32)
    bw = np.asarray(inputs["bw"], np.float32)

    I4 = np.eye(G, dtype=np.float32)

    def aug(gate_w, bias_row):
        m = np.zeros((2 * G + 1, P), np.float32)
        m[0:2 * G] = np.kron(I4, gate_w.T)
        m[2 * G] = np.tile(bias_row, G)
        return m.astype(np_bf16)

    bwt8 = np.outer(np.tile(bw, G),
                    np.arange(1, nsteps + 1)).astype(np.float32)
    bwt = np.zeros((P, nsteps), np.float32)
    for k in range(G):
        bwt[32 * k:32 * k + 2 * G] = bwt8
    common = {
        "w1d": np.ascontiguousarray(
            W1.T.reshape(2, P, H).transpose(1, 0, 2)).astype(np_bf16),
        "w2d": np.ascontiguousarray(
            W2.T.reshape(4, P, H).transpose(1, 0, 2)).astype(np_bf16),
        "wmd": np.ascontiguousarray(
            Wm.T.reshape(4, P, A).transpose(1, 0, 2)).astype(np_bf16),
        "b1d": np.ascontiguousarray(b1.reshape(4, P).T),
        "b2d": np.ascontiguousarray(b2.reshape(4, P).T),
        "bmd": bm.reshape(A, 1).copy(),
        "lrid": aug(w_ih[0:A], b_ih[0:A] + b_hh[0:A]),
        "luid": aug(w_ih[A:2 * A], b_ih[A:2 * A] + b_hh[A:2 * A]),
        "lnid": aug(w_ih[2 * A:3 * A], b_ih[2 * A:3 * A]),
        "lrhd": np.ascontiguousarray(np.kron(I4, w_hh[0:A].T)).astype(np_bf16),
        "luhd": np.ascontiguousarray(
            np.kron(I4, w_hh[A:2 * A].T)).astype(np_bf16),
        "lnhd": np.ascontiguousarray(
            np.kron(I4, w_hh[2 * A:3 * A].T)).astype(np_bf16),
        "lwd": _lwd(Ww),
        "zod": _zod(nsteps),
        "eyed": np.eye(P, dtype=np.float32).astype(np_bf16),
        "bnhd": np.tile(b_hh[2 * A:3 * A], G).reshape(P, 1).copy(),
        "bwtd": bwt,
    }
    return common


def kernel(**inputs) -> np.ndarray:
    global LAST_RESULT
    x = np.asarray(inputs["x"], dtype=np.float32)
    T = int(inputs["pred_length"])

    common = _prep_common(inputs, T)
    # x -> [P, 2, BC] per core: xd[p, kb, n] = x[n, kb*128+p]
    xT = np.ascontiguousarray(x.T.astype(np_bf16))      # [S, B]
    xT = xT.reshape(2, P, B)
    in_maps = []
    for i in range(NCORES):
        m = dict(common)
        m["xd"] = np.ascontiguousarray(
            xT[:, :, i * BC:(i + 1) * BC].transpose(1, 0, 2))
        in_maps.append(m)

    if T not in _CACHE:
        _CACHE[T] = _build(T)
    nc = _CACHE[T]
    res = run_bass_kernel_spmd(nc, in_maps, core_ids=list(range(NCORES)))
    LAST_RESULT = res
    parts = []
    for i in range(NCORES):
        o = np.asarray(res.results[i]["outd"]).astype(np.float32)
        # o[2g+j, t, m, n] -> out[m*2048 + g*512 + n, 2t+j]
        o = o.reshape(G, 2, T, NM, BT).transpose(3, 0, 4, 2, 1)
        parts.append(o.reshape(BC, 2 * T))
    return np.ascontiguousarray(np.concatenate(parts, axis=0))



# revision 18
# speedup vs baseline: 1.0335x; 1.0335x over previous
import sys

sys.path.insert(0, "/opt/trn_rl_repo")

import numpy as np

import concourse.bass as bass
import concourse.bacc as bacc
import concourse.mybir as mybir
from concourse.tile import TileContext
from concourse.bass_utils import run_bass_kernel_spmd

try:
    from ml_dtypes import bfloat16 as np_bf16
except ImportError:  # pragma: no cover
    import jax.numpy as _jnp

    np_bf16 = _jnp.bfloat16

P = 128          # partitions
BT = 512         # batch-tile (free dim) per matmul
G = 4            # batch groups packed into 128 partitions for the GRU
NCORES = 8
B, S, H, A = 131072, 256, 512, 32
BC = B // NCORES           # 16384 rows per core
MACRO = G * BT             # 2048 rows per GRU macro-tile
NM = BC // MACRO           # 8 macro-tiles per core
NG = BC // BT              # 32 MLP group-tiles per core

FP32 = mybir.dt.float32
BF16 = mybir.dt.bfloat16
AF = mybir.ActivationFunctionType
OP = mybir.AluOpType

_CACHE = {}

import os as _os
F_T0 = _os.environ.get("K_T0", "1") == "1"       # cheap t=0 cells
F_SDMA = _os.environ.get("K_SDMA", "1") == "1"   # split startup DMAs
F_NHALF = int(_os.environ.get("K_NHALF", "1"))   # weave column split
# which MLP f-tiles relu on DVE instead of Act
RELU_DVE = tuple(int(c) for c in _os.environ.get("K_RDVE", "12"))
DRAIN_ACT = _os.environ.get("K_DACT", "0") == "1"
Z0_ACT = _os.environ.get("K_Z0ACT", "0") == "1"
OMU_POOL = _os.environ.get("K_OMUP", "0") == "1"



def _build(nsteps: int) -> bass.Bass:
    nc = bacc.Bacc("TRN2", target_bir_lowering=False, debug=False,
                   num_devices=NCORES)

    xd = nc.dram_tensor("xd", [P, 2, BC], BF16, kind="ExternalInput")
    w1d = nc.dram_tensor("w1d", [P, 2, H], BF16, kind="ExternalInput")
    w2d = nc.dram_tensor("w2d", [P, 4, H], BF16, kind="ExternalInput")
    wmd = nc.dram_tensor("wmd", [P, 4, A], BF16, kind="ExternalInput")
    b1d = nc.dram_tensor("b1d", [P, 4], FP32, kind="ExternalInput")
    b2d = nc.dram_tensor("b2d", [P, 4], FP32, kind="ExternalInput")
    bmd = nc.dram_tensor("bmd", [A, 1], FP32, kind="ExternalInput")
    # augmented input-gate weights: rows 0-7 kron(I4, w_ih_gate.T), row 8 bias
    lrid = nc.dram_tensor("lrid", [2 * G + 1, P], BF16, kind="ExternalInput")
    luid = nc.dram_tensor("luid", [2 * G + 1, P], BF16, kind="ExternalInput")
    lnid = nc.dram_tensor("lnid", [2 * G + 1, P], BF16, kind="ExternalInput")
    lrhd = nc.dram_tensor("lrhd", [P, P], BF16, kind="ExternalInput")
    luhd = nc.dram_tensor("luhd", [P, P], BF16, kind="ExternalInput")
    lnhd = nc.dram_tensor("lnhd", [P, P], BF16, kind="ExternalInput")
    lwd = nc.dram_tensor("lwd", [P, NM, P], BF16, kind="ExternalInput")
    eyed = nc.dram_tensor("eyed", [P, P], BF16, kind="ExternalInput")
    bnhd = nc.dram_tensor("bnhd", [P, 1], FP32, kind="ExternalInput")
    bwtd = nc.dram_tensor("bwtd", [P, nsteps], FP32, kind="ExternalInput")
    # t=0 cells skip the i-side matmuls (wp=0): gate biases ride the Act
    # bias port instead; [P,1] per-partition columns
    brd = nc.dram_tensor("brd", [P, 1], FP32, kind="ExternalInput")
    bud = nc.dram_tensor("bud", [P, 1], FP32, kind="ExternalInput")
    bind = nc.dram_tensor("bind", [P, 1], FP32, kind="ExternalInput")
    # ones-row only: wp rows 0-7 of wpball are always drain-written before
    # read once t=0 stops reading them
    zod = nc.dram_tensor("zod", [1, nsteps + 1, NM, BT], BF16,
                         kind="ExternalInput")
    outd = nc.dram_tensor("outd", [2 * G, nsteps, NM, BT], BF16,
                          kind="ExternalOutput")

    with TileContext(nc) as tc:
        with (
            tc.tile_pool(name="const", bufs=1) as const,
            tc.tile_pool(name="state", bufs=1) as state,
            tc.tile_pool(name="xp", bufs=4) as xp,
            tc.tile_pool(name="actp", bufs=2) as actp,
            tc.tile_pool(name="gp", bufs=4) as gp,
            tc.tile_pool(name="psA", bufs=2, space="PSUM") as psA,
            tc.tile_pool(name="psB", bufs=2, space="PSUM") as psB,
            tc.tile_pool(name="psAcc", bufs=1, space="PSUM") as psAcc,
        ):
            # ---- MLP constants first so the first X DMA isn't queued
            # behind the GRU constants; w1 split in halves so the first L1
            # matmuls start as soon as the k=0 plane lands ----
            w1s = const.tile([P, 2, H], BF16)
            b1s = const.tile([P, 4], FP32)
            if F_SDMA:
                nc.sync.dma_start(w1s[:, 0:1, :], w1d[:, 0:1, :])
                nc.sync.dma_start(w1s[:, 1:2, :], w1d[:, 1:2, :])
            else:
                nc.sync.dma_start(w1s[:], w1d[:])
            nc.sync.dma_start(b1s[:], b1d[:])
            # L2/L3 constants on the Act-issued DMA queue: they transfer
            # in parallel with w1s/X on the SP queue during startup
            w2s = const.tile([P, 4, H], BF16)
            nc.scalar.dma_start(w2s[:], w2d[:])
            b2s = const.tile([P, 4], FP32)
            nc.scalar.dma_start(b2s[:], b2d[:])
            wms = const.tile([P, 4, A], BF16)
            nc.scalar.dma_start(wms[:], wmd[:])
            bms = const.tile([A, 1], FP32)
            nc.scalar.dma_start(bms[:], bmd[:])
            lris = const.tile([2 * G + 1, P], BF16)
            luis = const.tile([2 * G + 1, P], BF16)
            lnis = const.tile([2 * G + 1, P], BF16)
            lrhs = const.tile([P, P], BF16)
            luhs = const.tile([P, P], BF16)
            lnhs = const.tile([P, P], BF16)
            lws = const.tile([P, NM, P], BF16)
            eyes = const.tile([P, P], BF16)
            bnhs = const.tile([P, 1], FP32)
            bwts = const.tile([P, nsteps], FP32)
            brs = const.tile([P, 1], FP32)
            bus = const.tile([P, 1], FP32)
            bins = const.tile([P, 1], FP32)

            def load_gru_consts():
                nc.sync.dma_start(lris[:], lrid[:])
                nc.sync.dma_start(luis[:], luid[:])
                nc.sync.dma_start(lnis[:], lnid[:])
                nc.sync.dma_start(lrhs[:], lrhd[:])
                nc.sync.dma_start(luhs[:], luhd[:])
                nc.sync.dma_start(lnhs[:], lnhd[:])
                nc.sync.dma_start(lws[:], lwd[:])
                nc.sync.dma_start(eyes[:], eyed[:])
                nc.sync.dma_start(bnhs[:], bnhd[:])
                nc.sync.dma_start(bwts[:], bwtd[:])
                nc.sync.dma_start(brs[:], brd[:])
                nc.sync.dma_start(bus[:], bud[:])
                nc.sync.dma_start(bins[:], bind[:])
                nc.sync.dma_start(wpball[2 * G:2 * G + 1, :, :, :], zod[:])

            # ---- persistent state ----
            # double-buffered by step parity: the Zn write must not WAR-wait
            # on the slow Pool-engine reads of the previous value
            Zb2 = []
            for par in range(2):
                row = []
                for m in range(NM):
                    z = state.tile([P, BT], BF16, tag=f"Z{par}_{m}",
                                   name=f"Zt{par}_{m}")
                    row.append(z)
                Zb2.append(row)
            Zb = Zb2[0]
            # all wp steps in one tile: rows 0-7 wp, row 8 ones (bias matmul);
            # slice [:, t, m, :] is macro m's wp after step t-1 (t=0 is init)
            wpball = state.tile([2 * G + 1, nsteps + 1, NM, BT], BF16,
                                tag="wpball", name="wpball")
            # wp accumulators: two PSUM banks; macro m lives at the
            # 32-aligned offset 32*(m%4) of bank m//4 (engine reads must
            # start on a 32-partition boundary)
            paccA = psAcc.tile([P, BT], FP32, tag="pA", name="paccA")
            paccB = psAcc.tile([P, BT], FP32, tag="pB", name="paccB")

            # ---- fused wavefront: MLP macros interleaved with GRU cells ----
            def relu_out(dst, src, bias, which):
                # src is PSUM: only Act and DVE may read it (GPSIMD cannot)
                if which == 0:
                    nc.scalar.activation(dst, src, AF.Relu, bias=bias)
                else:
                    nc.vector.tensor_scalar(dst, src, bias, 0.0, OP.add,
                                            OP.max)

            def mlp_pair(g0, first=False):
                # two batch groups in the free-dim halves of each PSUM tile:
                # one [128,1024] relu (same per-partition bias) serves both
                Xs = []
                for g in (g0, g0 + 1):
                    X = xp.tile([P, 2, BT], BF16, tag="X", name="X")
                    Xs.append(X)
                if first and F_SDMA:
                    # startup: X on the idle Pool DMA queue (k=0 planes
                    # first) so the first matmuls aren't serialized behind
                    # the w1 transfer on the SP queue
                    for k in range(2):
                        for j, g in enumerate((g0, g0 + 1)):
                            nc.gpsimd.dma_start(
                                Xs[j][:, k:k + 1, :],
                                xd[:, k:k + 1, g * BT:(g + 1) * BT])
                else:
                    for j, g in enumerate((g0, g0 + 1)):
                        nc.sync.dma_start(Xs[j][:],
                                          xd[:, :, g * BT:(g + 1) * BT])
                H1 = actp.tile([P, 4, 2, BT], BF16, tag="H1", name="H1")
                H2 = actp.tile([P, 4, 2, BT], BF16, tag="H2", name="H2")
                for f in range(4):
                    ps = psA.tile([P, 2, BT], FP32, tag="A", name="psL1")
                    for k in range(2):
                        for j in range(2):
                            nc.tensor.matmul(ps[:, j, :],
                                             w1s[:, k, f * P:(f + 1) * P],
                                             Xs[j][:, k, :], start=(k == 0),
                                             stop=(k == 1),
                                             skip_group_check=True)
                    relu_out(H1[:, f, :, :], ps[:], b1s[:, f:f + 1],
                             0 if f not in RELU_DVE else 1)
                for f in range(4):
                    ps = psA.tile([P, 2, BT], FP32, tag="A", name="psL2")
                    for k in range(4):
                        for j in range(2):
                            nc.tensor.matmul(ps[:, j, :],
                                             w2s[:, k, f * P:(f + 1) * P],
                                             H1[:, k, j, :], start=(k == 0),
                                             stop=(k == 3),
                                             skip_group_check=True)
                    relu_out(H2[:, f, :, :], ps[:], b2s[:, f:f + 1],
                             0 if f not in RELU_DVE else 1)
                ps3 = [psB.tile([A, BT], FP32, tag="B", name="psL3")[:]
                       for _ in range(2)]
                for k in range(4):
                    for j in range(2):
                        nc.tensor.matmul(ps3[j], wms[:, k, :],
                                         H2[:, k, j, :], start=(k == 0),
                                         stop=(k == 3), skip_group_check=True)
                for j in range(2):
                    g = g0 + j
                    m, q = g // G, g % G
                    if j == 0 and not Z0_ACT:
                        nc.vector.tensor_scalar_add(
                            Zb[m][q * A:(q + 1) * A, :], ps3[j],
                            bms[:, 0:1])
                    else:
                        nc.scalar.activation(Zb[m][q * A:(q + 1) * A, :],
                                             ps3[j], AF.Identity,
                                             bias=bms[:, 0:1])

            cellno = [0]

            class Cell:
                __slots__ = ("t", "m", "c0", "cw", "par", "t0c", "fast",
                             "pB", "RU", "T1", "OMU", "P1", "zsl", "znew",
                             "wprev")

            # --- software-pipelined GRU cell: three stages, each emitted a
            # scheduling slot apart so no PE instruction ever queues right
            # behind the elementwise chain that feeds it ---
            def stage_gates(t, m, fast=False, c0=0, cw=BT):
                c = Cell()
                c.t, c.m, c.c0, c.cw, c.fast = t, m, c0, cw, fast
                c.par = cellno[0] % 2
                cellno[0] += 1
                # t=0: wp is identically zero, so every i-side matmul drops
                # out; gate biases ride the Act bias port instead
                c.t0c = (t == 0) and F_T0
                c.wprev = wpball[:, t, m, c0:c0 + cw]
                c.zsl = Zb2[t % 2][m][:, c0:c0 + cw]
                c.znew = Zb2[(t + 1) % 2][m][:, c0:c0 + cw]
                psRU = psA.tile([P, 2, cw], FP32, tag="A", name="psRU")
                nc.tensor.matmul(psRU[:, 0, :], lrhs[:], c.zsl,
                                 start=True, stop=c.t0c)
                if not c.t0c:
                    nc.tensor.matmul(psRU[:, 0, :], lris[:], c.wprev,
                                     start=False, stop=True)
                nc.tensor.matmul(psRU[:, 1, :], luhs[:], c.zsl,
                                 start=True, stop=c.t0c)
                if not c.t0c:
                    nc.tensor.matmul(psRU[:, 1, :], luis[:], c.wprev,
                                     start=False, stop=True)
                c.pB = psB.tile([P, cw], FP32, tag="B", name="pB")
                nc.tensor.matmul(c.pB[:], lnhs[:], c.zsl,
                                 start=True, stop=True, skip_group_check=True)
                c.RU = gp.tile([P, 2, cw], BF16, tag="RU", name="RU")
                if c.t0c:
                    nc.scalar.activation(c.RU[:, 0, :], psRU[:, 0, :],
                                         AF.Sigmoid, bias=brs[:, 0:1])
                    nc.scalar.activation(c.RU[:, 1, :], psRU[:, 1, :],
                                         AF.Sigmoid, bias=bus[:, 0:1])
                else:
                    nc.scalar.activation(c.RU[:], psRU[:], AF.Sigmoid)
                HN = gp.tile([P, cw], BF16, tag="HN", name="HN")
                nc.vector.tensor_scalar_add(HN[:], c.pB[:], bnhs[:, 0:1])
                c.T1 = gp.tile([P, cw], BF16, tag="T1", name="T1")
                nc.vector.tensor_tensor(c.T1[:], c.RU[:, 0, :], HN[:],
                                        OP.mult)
                c.P1 = c.OMU = None
                if fast:
                    c.P1 = gp.tile([P, cw], BF16, tag="D", name="P1")
                    nc.gpsimd.tensor_tensor(c.P1[:], c.RU[:, 1, :], c.zsl,
                                            OP.mult)
                    c.OMU = gp.tile([P, cw], BF16, tag="E", name="OMU")
                    omu_eng = nc.gpsimd if OMU_POOL else nc.vector
                    omu_eng.tensor_scalar(c.OMU[:], c.RU[:, 1, :], -1.0,
                                          1.0, OP.mult, OP.add)
                return c

            def stage_n(c):
                NT = gp.tile([P, c.cw], BF16, tag="NT", name="NT")
                if c.t0c:
                    # i_n = b_ihn at t=0: tanh reads T1 straight from SBUF
                    nc.scalar.activation(NT[:], c.T1[:], AF.Tanh,
                                         bias=bins[:, 0:1])
                else:
                    # reuse the bank: fresh group = lni@wp + T1 via identity
                    nc.tensor.matmul(c.pB[:], lnis[:], c.wprev,
                                     start=True, stop=False,
                                     skip_group_check=True)
                    nc.tensor.matmul(c.pB[:], eyes[:], c.T1[:],
                                     start=False, stop=True,
                                     skip_group_check=True)
                    nc.scalar.activation(NT[:], c.pB[:], AF.Tanh)
                if c.fast:
                    P2 = gp.tile([P, c.cw], BF16, tag="P2", name="P2")
                    nc.vector.tensor_tensor(P2[:], c.OMU[:], NT[:], OP.mult)
                    nc.vector.tensor_tensor(c.znew, c.P1[:], P2[:], OP.add)
                else:
                    D = gp.tile([P, c.cw], BF16, tag="D", name="D")
                    nc.gpsimd.tensor_tensor(D[:], c.zsl, NT[:], OP.subtract)
                    E = gp.tile([P, c.cw], BF16, tag="E", name="E")
                    if c.par == 0:
                        nc.gpsimd.tensor_tensor(E[:], c.RU[:, 1, :], D[:],
                                                OP.mult)
                    else:
                        nc.vector.tensor_tensor(E[:], c.RU[:, 1, :], D[:],
                                                OP.mult)
                    nc.vector.tensor_tensor(c.znew, E[:], NT[:], OP.add)

            def stage_w(c):
                pacc = paccA if c.m < G else paccB
                off = 32 * (c.m % G)
                nc.tensor.matmul(pacc[:, c.c0:c.c0 + c.cw], lws[:, c.m, :],
                                 c.znew,
                                 start=(c.t == 0 and c.m % G == 0),
                                 stop=(c.t == nsteps - 1 and
                                       c.m % G == G - 1 and
                                       c.c0 + c.cw == BT),
                                 skip_group_check=True)
                if DRAIN_ACT or (c.fast and c.par == 0):
                    nc.scalar.activation(
                        wpball[0:2 * G, c.t + 1, c.m, c.c0:c.c0 + c.cw],
                        pacc[off:off + 2 * G, c.c0:c.c0 + c.cw],
                        AF.Identity,
                        bias=bwts[off:off + 2 * G, c.t:c.t + 1])
                else:
                    nc.vector.tensor_scalar_add(
                        wpball[0:2 * G, c.t + 1, c.m, c.c0:c.c0 + c.cw],
                        pacc[off:off + 2 * G, c.c0:c.c0 + c.cw],
                        bwts[off:off + 2 * G, c.t:c.t + 1])

            # pipeline state
            next_t = [0] * NM
            ndone = [-1] * NM      # highest t with stage_n emitted
            wdone = [-1] * NM      # highest t with stage_w emitted
            pend_n = []            # gates emitted, stage_n pending
            pend_w = []            # stage_n emitted, stage_w pending
            dma_t = [0]

            def flush_dmas(slack):
                while (dma_t[0] < nsteps
                       and all(w >= dma_t[0] + slack for w in wdone)):
                    t0 = dma_t[0]
                    nc.sync.dma_start(outd[:, t0, :, :],
                                      wpball[0:2 * G, t0 + 1, :, :])
                    dma_t[0] = t0 + 1

            def batch(mm_max, dcap, fast=False):
                nsnap = list(ndone)
                wsnap = list(wdone)
                for c in pend_n:
                    stage_n(c)
                    ndone[c.m] = c.t
                moved = pend_n[:]
                pend_n.clear()
                old_w = pend_w[:]
                pend_w.clear()
                specs = []
                for m in range(mm_max):
                    t = next_t[m]
                    if (t < nsteps and t + m <= dcap and len(specs) < 4
                            and (t == 0 or (nsnap[m] >= t - 1
                                            and wsnap[m] >= t - 1))):
                        specs.append((t, m))
                        next_t[m] = t + 1
                # interleave new gates with old w-stages so every lws has
                # >= one gates-group of PE work between it and its znew
                gi = wi = 0
                while gi < len(specs) or wi < len(old_w):
                    if gi < len(specs):
                        t, m = specs[gi]
                        gi += 1
                        pend_n.append(stage_gates(t, m, fast=fast))
                    if wi < len(old_w):
                        c = old_w[wi]
                        wi += 1
                        stage_w(c)
                        wdone[c.m] = c.t
                pend_w.extend(moved)
                flush_dmas(1)

            for s in range(NM):
                batch(s, 2 * s - 2)
                mlp_pair(G * s, first=(s == 0))
                if s == 0:
                    load_gru_consts()
                batch(s, 2 * s - 1)
                mlp_pair(G * s + 2)
            # drain weave: greedy most-starved chain, software-pipelined,
            # with chain cadence >= 3 slots enforced via the stage snapshots
            SP_W = 3
            lastg = [-SP_W] * NM
            pos = 0
            while (any(t < nsteps for t in next_t) or pend_n or pend_w):
                nsnap = list(ndone)
                wsnap = list(wdone)
                for c in pend_n:
                    stage_n(c)
                    ndone[c.m] = c.t
                moved = pend_n[:]
                pend_n.clear()

                def elig(m, snapn, snapw):
                    t = next_t[m]
                    return (t < nsteps
                            and (t == 0 or (snapn[m] >= t - 1
                                            and snapw[m] >= t - 1)))

                cand = [m for m in range(NM)
                        if elig(m, nsnap, wsnap) and pos - lastg[m] >= SP_W]
                if not cand:
                    cand = [m for m in range(NM) if elig(m, nsnap, wsnap)]
                if not cand and not moved and not pend_w:
                    cand = [m for m in range(NM)
                            if elig(m, ndone, wdone)]
                if cand:
                    m = max(cand, key=lambda mm: nsteps - next_t[mm])
                    t = next_t[m]
                    next_t[m] = t + 1
                    pend_n.append(stage_gates(t, m, fast=True))
                    lastg[m] = pos
                for c in pend_w:
                    stage_w(c)
                    wdone[c.m] = c.t
                pend_w.clear()
                pend_w.extend(moved)
                pos += 1
                flush_dmas(1)
            while dma_t[0] < nsteps:
                t0 = dma_t[0]
                nc.sync.dma_start(outd[:, t0, :, :],
                                  wpball[0:2 * G, t0 + 1, :, :])
                dma_t[0] = t0 + 1
    nc.compile()
    return nc


LAST_RESULT = None


def _lwd(Ww):
    I4 = np.eye(G, dtype=np.float32)
    blk = np.kron(I4, Ww.T)                      # [128, 8]
    out = np.zeros((P, NM, P), np.float32)
    for m in range(NM):
        off = 32 * (m % G)
        out[:, m, off:off + 2 * G] = blk
    return out.astype(np_bf16)


def _zod(nsteps):
    # ones-row only: wp rows are always drain-written before read
    return np.ones((1, nsteps + 1, NM, BT), np.float32).astype(np_bf16)


def _prep_common(inputs, nsteps):
    W1 = np.asarray(inputs["W1"], np.float32)
    b1 = np.asarray(inputs["b1"], np.float32)
    W2 = np.asarray(inputs["W2"], np.float32)
    b2 = np.asarray(inputs["b2"], np.float32)
    Wm = np.asarray(inputs["Wm"], np.float32)
    bm = np.asarray(inputs["bm"], np.float32)
    w_ih = np.asarray(inputs["w_ih"], np.float32)
    w_hh = np.asarray(inputs["w_hh"], np.float32)
    b_ih = np.asarray(inputs["b_ih"], np.float32)
    b_hh = np.asarray(inputs["b_hh"], np.float32)
    Ww = np.asarray(inputs["Ww"], np.float32)
    bw = np.asarray(inputs["bw"], np.float32)

    I4 = np.eye(G, dtype=np.float32)

    def aug(gate_w, bias_row):
        m = np.zeros((2 * G + 1, P), np.float32)
        m[0:2 * G] = np.kron(I4, gate_w.T)
        m[2 * G] = np.tile(bias_row, G)
        return m.astype(np_bf16)

    bwt8 = np.outer(np.tile(bw, G),
                    np.arange(1, nsteps + 1)).astype(np.float32)
    bwt = np.zeros((P, nsteps), np.float32)
    for k in range(G):
        bwt[32 * k:32 * k + 2 * G] = bwt8
    common = {
        "w1d": np.ascontiguousarray(
            W1.T.reshape(2, P, H).transpose(1, 0, 2)).astype(np_bf16),
        "w2d": np.ascontiguousarray(
            W2.T.reshape(4, P, H).transpose(1, 0, 2)).astype(np_bf16),
        "wmd": np.ascontiguousarray(
            Wm.T.reshape(4, P, A).transpose(1, 0, 2)).astype(np_bf16),
        "b1d": np.ascontiguousarray(b1.reshape(4, P).T),
        "b2d": np.ascontiguousarray(b2.reshape(4, P).T),
        "bmd": bm.reshape(A, 1).copy(),
        "lrid": aug(w_ih[0:A], b_ih[0:A] + b_hh[0:A]),
        "luid": aug(w_ih[A:2 * A], b_ih[A:2 * A] + b_hh[A:2 * A]),
        "lnid": aug(w_ih[2 * A:3 * A], b_ih[2 * A:3 * A]),
        "lrhd": np.ascontiguousarray(np.kron(I4, w_hh[0:A].T)).astype(np_bf16),
        "luhd": np.ascontiguousarray(
            np.kron(I4, w_hh[A:2 * A].T)).astype(np_bf16),
        "lnhd": np.ascontiguousarray(
            np.kron(I4, w_hh[2 * A:3 * A].T)).astype(np_bf16),
        "lwd": _lwd(Ww),
        "zod": _zod(nsteps),
        "eyed": np.eye(P, dtype=np.float32).astype(np_bf16),
        "bnhd": np.tile(b_hh[2 * A:3 * A], G).reshape(P, 1).copy(),
        "bwtd": bwt,
        "brd": np.tile(b_ih[0:A] + b_hh[0:A], G).reshape(P, 1).copy(),
        "bud": np.tile(b_ih[A:2 * A] + b_hh[A:2 * A], G).reshape(P, 1).copy(),
        "bind": np.tile(b_ih[2 * A:3 * A], G).reshape(P, 1).copy(),
    }
    return common


def kernel(**inputs) -> np.ndarray:
    global LAST_RESULT
    x = np.asarray(inputs["x"], dtype=np.float32)
    T = int(inputs["pred_length"])

    common = _prep_common(inputs, T)
    # x -> [P, 2, BC] per core: xd[p, kb, n] = x[n, kb*128+p]
    xT = np.ascontiguousarray(x.T.astype(np_bf16))      # [S, B]
    xT = xT.reshape(2, P, B)
    in_maps = []
    for i in range(NCORES):
        m = dict(common)
        m["xd"] = np.ascontiguousarray(
            xT[:, :, i * BC:(i + 1) * BC].transpose(1, 0, 2))
        in_maps.append(m)

    if T not in _CACHE:
        _CACHE[T] = _build(T)
    nc = _CACHE[T]
    res = run_bass_kernel_spmd(nc, in_maps, core_ids=list(range(NCORES)))
    LAST_RESULT = res
    parts = []
    for i in range(NCORES):
        o = np.asarray(res.results[i]["outd"]).astype(np.float32)
        # o[2g+j, t, m, n] -> out[m*2048 + g*512 + n, 2t+j]
        o = o.reshape(G, 2, T, NM, BT).transpose(3, 0, 4, 2, 1)
        parts.append(o.reshape(BC, 2 * T))
    return np.ascontiguousarray(np.concatenate(parts, axis=0))



# revision 31
# speedup vs baseline: 1.0482x; 1.0142x over previous
import sys

sys.path.insert(0, "/opt/trn_rl_repo")

import numpy as np

import concourse.bass as bass
import concourse.bacc as bacc
import concourse.mybir as mybir
from concourse.tile import TileContext
from concourse.bass_utils import run_bass_kernel_spmd

try:
    from ml_dtypes import bfloat16 as np_bf16
except ImportError:  # pragma: no cover
    import jax.numpy as _jnp

    np_bf16 = _jnp.bfloat16

P = 128          # partitions
BT = 512         # batch-tile (free dim) per matmul
G = 4            # batch groups packed into 128 partitions for the GRU
NCORES = 8
B, S, H, A = 131072, 256, 512, 32
BC = B // NCORES           # 16384 rows per core
MACRO = G * BT             # 2048 rows per GRU macro-tile
NM = BC // MACRO           # 8 macro-tiles per core
NG = BC // BT              # 32 MLP group-tiles per core

FP32 = mybir.dt.float32
BF16 = mybir.dt.bfloat16
AF = mybir.ActivationFunctionType
OP = mybir.AluOpType

_CACHE = {}

import os as _os
F_T0 = _os.environ.get("K_T0", "1") == "1"       # cheap t=0 cells
F_SDMA = _os.environ.get("K_SDMA", "1") == "1"   # split startup DMAs
F_NHALF = int(_os.environ.get("K_NHALF", "1"))   # weave column split
# which MLP f-tiles relu on DVE instead of Act
RELU_DVE = tuple(int(c) for c in _os.environ.get("K_RDVE", "12"))
DRAIN_ACT = _os.environ.get("K_DACT", "0") == "1"
Z0_ACT = _os.environ.get("K_Z0ACT", "0") == "1"
OMU_POOL = _os.environ.get("K_OMUP", "0") == "1"
ZN_POOL = _os.environ.get("K_ZNP", "0") == "1"   # dense znew on Pool
HN_ACT = _os.environ.get("K_HNA", "0") == "1"    # dense HN on Act
T9_STORE = _os.environ.get("K_T9", "1") == "1"   # per-macro last-step store
WMID = _os.environ.get("K_WMID", "1") == "1"     # w-stages mid-pair
XSCAL = _os.environ.get("K_XSC", "2")            # first X: 1=scalar, 2=gpsimd



def _build(nsteps: int) -> bass.Bass:
    nc = bacc.Bacc("TRN2", target_bir_lowering=False, debug=False,
                   num_devices=NCORES)

    xd = nc.dram_tensor("xd", [P, 2, BC], BF16, kind="ExternalInput")
    w1d = nc.dram_tensor("w1d", [P, 2, H], BF16, kind="ExternalInput")
    w2d = nc.dram_tensor("w2d", [P, 4, H], BF16, kind="ExternalInput")
    wmd = nc.dram_tensor("wmd", [P, 4, A], BF16, kind="ExternalInput")
    b1d = nc.dram_tensor("b1d", [P, 4], FP32, kind="ExternalInput")
    b2d = nc.dram_tensor("b2d", [P, 4], FP32, kind="ExternalInput")
    bmd = nc.dram_tensor("bmd", [A, 1], FP32, kind="ExternalInput")
    # augmented input-gate weights: rows 0-7 kron(I4, w_ih_gate.T), row 8 bias
    lrid = nc.dram_tensor("lrid", [2 * G + 1, P], BF16, kind="ExternalInput")
    luid = nc.dram_tensor("luid", [2 * G + 1, P], BF16, kind="ExternalInput")
    lnid = nc.dram_tensor("lnid", [2 * G + 1, P], BF16, kind="ExternalInput")
    lrhd = nc.dram_tensor("lrhd", [P, P], BF16, kind="ExternalInput")
    luhd = nc.dram_tensor("luhd", [P, P], BF16, kind="ExternalInput")
    lnhd = nc.dram_tensor("lnhd", [P, P], BF16, kind="ExternalInput")
    lwd = nc.dram_tensor("lwd", [P, NM, P], BF16, kind="ExternalInput")
    eyed = nc.dram_tensor("eyed", [P, P], BF16, kind="ExternalInput")
    bnhd = nc.dram_tensor("bnhd", [P, 1], FP32, kind="ExternalInput")
    bwtd = nc.dram_tensor("bwtd", [P, nsteps], FP32, kind="ExternalInput")
    # t=0 cells skip the i-side matmuls (wp=0): gate biases ride the Act
    # bias port instead; [P,1] per-partition columns
    brd = nc.dram_tensor("brd", [P, 1], FP32, kind="ExternalInput")
    bud = nc.dram_tensor("bud", [P, 1], FP32, kind="ExternalInput")
    bind = nc.dram_tensor("bind", [P, 1], FP32, kind="ExternalInput")
    # ones-row only: wp rows 0-7 of wpball are always drain-written before
    # read once t=0 stops reading them
    zod = nc.dram_tensor("zod", [1, nsteps + 1, NM, BT], BF16,
                         kind="ExternalInput")
    outd = nc.dram_tensor("outd", [2 * G, nsteps, NM, BT], BF16,
                          kind="ExternalOutput")

    with TileContext(nc) as tc:
        with (
            tc.tile_pool(name="const", bufs=1) as const,
            tc.tile_pool(name="state", bufs=1) as state,
            tc.tile_pool(name="xp", bufs=4) as xp,
            tc.tile_pool(name="actp", bufs=2) as actp,
            tc.tile_pool(name="gp", bufs=int(_os.environ.get("K_GP", "6"))) as gp,
            tc.tile_pool(name="psA", bufs=2, space="PSUM") as psA,
            tc.tile_pool(name="psB", bufs=2, space="PSUM") as psB,
            tc.tile_pool(name="psAcc", bufs=1, space="PSUM") as psAcc,
        ):
            # ---- MLP constants first so the first X DMA isn't queued
            # behind the GRU constants; w1 split in halves so the first L1
            # matmuls start as soon as the k=0 plane lands ----
            w1s = const.tile([P, 2, H], BF16)
            b1s = const.tile([P, 4], FP32)
            if F_SDMA:
                nc.sync.dma_start(w1s[:, 0:1, :], w1d[:, 0:1, :])
                nc.sync.dma_start(w1s[:, 1:2, :], w1d[:, 1:2, :])
            else:
                nc.sync.dma_start(w1s[:], w1d[:])
            nc.sync.dma_start(b1s[:], b1d[:])
            # first pair's X rides the Act DMA queue ahead of w2s: both
            # k=0 planes first so the first L1 matmuls can start early
            firstX = []
            if F_SDMA and XSCAL == "1":
                for g in range(2):
                    firstX.append(xp.tile([P, 2, BT], BF16, tag="X",
                                          name="X"))
                for k in range(2):
                    for j in range(2):
                        nc.scalar.dma_start(
                            firstX[j][:, k:k + 1, :],
                            xd[:, k:k + 1, j * BT:(j + 1) * BT])
            # L2/L3 constants on the Act-issued DMA queue: they transfer
            # in parallel with w1s/X on the SP queue during startup
            w2s = const.tile([P, 4, H], BF16)
            nc.scalar.dma_start(w2s[:], w2d[:])
            b2s = const.tile([P, 4], FP32)
            nc.scalar.dma_start(b2s[:], b2d[:])
            wms = const.tile([P, 4, A], BF16)
            nc.scalar.dma_start(wms[:], wmd[:])
            bms = const.tile([A, 1], FP32)
            nc.scalar.dma_start(bms[:], bmd[:])
            lris = const.tile([2 * G + 1, P], BF16)
            luis = const.tile([2 * G + 1, P], BF16)
            lnis = const.tile([2 * G + 1, P], BF16)
            lrhs = const.tile([P, P], BF16)
            luhs = const.tile([P, P], BF16)
            lnhs = const.tile([P, P], BF16)
            lws = const.tile([P, NM, P], BF16)
            eyes = const.tile([P, P], BF16)
            bnhs = const.tile([P, 1], FP32)
            bwts = const.tile([P, nsteps], FP32)
            brs = const.tile([P, 1], FP32)
            bus = const.tile([P, 1], FP32)
            bins = const.tile([P, 1], FP32)

            def load_gru_consts():
                nc.sync.dma_start(lris[:], lrid[:])
                nc.sync.dma_start(luis[:], luid[:])
                nc.sync.dma_start(lnis[:], lnid[:])
                nc.sync.dma_start(lrhs[:], lrhd[:])
                nc.sync.dma_start(luhs[:], luhd[:])
                nc.sync.dma_start(lnhs[:], lnhd[:])
                nc.sync.dma_start(lws[:], lwd[:])
                nc.sync.dma_start(eyes[:], eyed[:])
                nc.sync.dma_start(bnhs[:], bnhd[:])
                nc.sync.dma_start(bwts[:], bwtd[:])
                nc.sync.dma_start(brs[:], brd[:])
                nc.sync.dma_start(bus[:], bud[:])
                nc.sync.dma_start(bins[:], bind[:])
                nc.sync.dma_start(wpball[2 * G:2 * G + 1, :, :, :], zod[:])

            # ---- persistent state ----
            # double-buffered by step parity: the Zn write must not WAR-wait
            # on the slow Pool-engine reads of the previous value
            Zb2 = []
            for par in range(2):
                row = []
                for m in range(NM):
                    z = state.tile([P, BT], BF16, tag=f"Z{par}_{m}",
                                   name=f"Zt{par}_{m}")
                    row.append(z)
                Zb2.append(row)
            Zb = Zb2[0]
            # all wp steps in one tile: rows 0-7 wp, row 8 ones (bias matmul);
            # slice [:, t, m, :] is macro m's wp after step t-1 (t=0 is init)
            wpball = state.tile([2 * G + 1, nsteps + 1, NM, BT], BF16,
                                tag="wpball", name="wpball")
            # wp accumulators: two PSUM banks; macro m lives at the
            # 32-aligned offset 32*(m%4) of bank m//4 (engine reads must
            # start on a 32-partition boundary)
            paccA = psAcc.tile([P, BT], FP32, tag="pA", name="paccA")
            paccB = psAcc.tile([P, BT], FP32, tag="pB", name="paccB")

            # ---- fused wavefront: MLP macros interleaved with GRU cells ----
            def relu_out(dst, src, bias, which):
                # src is PSUM: only Act and DVE may read it (GPSIMD cannot)
                if which == 0:
                    nc.scalar.activation(dst, src, AF.Relu, bias=bias)
                else:
                    nc.vector.tensor_scalar(dst, src, bias, 0.0, OP.add,
                                            OP.max)

            def mlp_pair(g0, first=False, mid_cb=None):
                # two batch groups in the free-dim halves of each PSUM tile:
                # one [128,1024] relu (same per-partition bias) serves both
                if first and F_SDMA and XSCAL == "1":
                    Xs = firstX
                elif first and F_SDMA and XSCAL in ("2", "3"):
                    q = nc.gpsimd if XSCAL == "2" else nc.sync
                    Xs = []
                    for g in (g0, g0 + 1):
                        Xs.append(xp.tile([P, 2, BT], BF16, tag="X",
                                          name="X"))
                    for k in range(2):
                        for j in range(2):
                            q.dma_start(
                                Xs[j][:, k:k + 1, :],
                                xd[:, k:k + 1, j * BT:(j + 1) * BT])
                else:
                    Xs = []
                    for g in (g0, g0 + 1):
                        X = xp.tile([P, 2, BT], BF16, tag="X", name="X")
                        Xs.append(X)
                    for j, g in enumerate((g0, g0 + 1)):
                        nc.sync.dma_start(Xs[j][:],
                                          xd[:, :, g * BT:(g + 1) * BT])
                H1 = actp.tile([P, 4, 2, BT], BF16, tag="H1", name="H1")
                H2 = actp.tile([P, 4, 2, BT], BF16, tag="H2", name="H2")
                for f in range(4):
                    ps = psA.tile([P, 2, BT], FP32, tag="A", name="psL1")
                    for k in range(2):
                        for j in range(2):
                            nc.tensor.matmul(ps[:, j, :],
                                             w1s[:, k, f * P:(f + 1) * P],
                                             Xs[j][:, k, :], start=(k == 0),
                                             stop=(k == 1),
                                             skip_group_check=True)
                    relu_out(H1[:, f, :, :], ps[:], b1s[:, f:f + 1],
                             0 if f not in RELU_DVE else 1)
                if mid_cb is not None:
                    mid_cb()
                for f in range(4):
                    ps = psA.tile([P, 2, BT], FP32, tag="A", name="psL2")
                    for k in range(4):
                        for j in range(2):
                            nc.tensor.matmul(ps[:, j, :],
                                             w2s[:, k, f * P:(f + 1) * P],
                                             H1[:, k, j, :], start=(k == 0),
                                             stop=(k == 3),
                                             skip_group_check=True)
                    relu_out(H2[:, f, :, :], ps[:], b2s[:, f:f + 1],
                             0 if f not in RELU_DVE else 1)
                ps3 = [psB.tile([A, BT], FP32, tag="B", name="psL3")[:]
                       for _ in range(2)]
                for k in range(4):
                    for j in range(2):
                        nc.tensor.matmul(ps3[j], wms[:, k, :],
                                         H2[:, k, j, :], start=(k == 0),
                                         stop=(k == 3), skip_group_check=True)
                for j in range(2):
                    g = g0 + j
                    m, q = g // G, g % G
                    if j == 0 and not Z0_ACT:
                        nc.vector.tensor_scalar_add(
                            Zb[m][q * A:(q + 1) * A, :], ps3[j],
                            bms[:, 0:1])
                    else:
                        nc.scalar.activation(Zb[m][q * A:(q + 1) * A, :],
                                             ps3[j], AF.Identity,
                                             bias=bms[:, 0:1])

            cellno = [0]

            class Cell:
                __slots__ = ("t", "m", "c0", "cw", "par", "t0c", "fast",
                             "pB", "RU", "T1", "OMU", "P1", "zsl", "znew",
                             "wprev")

            # --- software-pipelined GRU cell: three stages, each emitted a
            # scheduling slot apart so no PE instruction ever queues right
            # behind the elementwise chain that feeds it ---
            def stage_gates(t, m, fast=False, c0=0, cw=BT):
                c = Cell()
                c.t, c.m, c.c0, c.cw, c.fast = t, m, c0, cw, fast
                c.par = cellno[0] % 2
                cellno[0] += 1
                # t=0: wp is identically zero, so every i-side matmul drops
                # out; gate biases ride the Act bias port instead
                c.t0c = (t == 0) and F_T0
                c.wprev = wpball[:, t, m, c0:c0 + cw]
                c.zsl = Zb2[t % 2][m][:, c0:c0 + cw]
                c.znew = Zb2[(t + 1) % 2][m][:, c0:c0 + cw]
                psRU = psA.tile([P, 2, cw], FP32, tag="A", name="psRU")
                nc.tensor.matmul(psRU[:, 0, :], lrhs[:], c.zsl,
                                 start=True, stop=c.t0c)
                if not c.t0c:
                    nc.tensor.matmul(psRU[:, 0, :], lris[:], c.wprev,
                                     start=False, stop=True)
                nc.tensor.matmul(psRU[:, 1, :], luhs[:], c.zsl,
                                 start=True, stop=c.t0c)
                if not c.t0c:
                    nc.tensor.matmul(psRU[:, 1, :], luis[:], c.wprev,
                                     start=False, stop=True)
                c.pB = psB.tile([P, cw], FP32, tag="B", name="pB")
                nc.tensor.matmul(c.pB[:], lnhs[:], c.zsl,
                                 start=True, stop=True, skip_group_check=True)
                c.RU = gp.tile([P, 2, cw], BF16, tag="RU", name="RU")
                if c.t0c:
                    nc.scalar.activation(c.RU[:, 0, :], psRU[:, 0, :],
                                         AF.Sigmoid, bias=brs[:, 0:1])
                    nc.scalar.activation(c.RU[:, 1, :], psRU[:, 1, :],
                                         AF.Sigmoid, bias=bus[:, 0:1])
                else:
                    nc.scalar.activation(c.RU[:], psRU[:], AF.Sigmoid)
                HN = gp.tile([P, cw], BF16, tag="HN", name="HN")
                if HN_ACT and not fast:
                    nc.scalar.activation(HN[:], c.pB[:], AF.Identity,
                                         bias=bnhs[:, 0:1])
                else:
                    nc.vector.tensor_scalar_add(HN[:], c.pB[:], bnhs[:, 0:1])
                c.T1 = gp.tile([P, cw], BF16, tag="T1", name="T1")
                nc.vector.tensor_tensor(c.T1[:], c.RU[:, 0, :], HN[:],
                                        OP.mult)
                c.P1 = c.OMU = None
                if fast:
                    c.P1 = gp.tile([P, cw], BF16, tag="D", name="P1")
                    nc.gpsimd.tensor_tensor(c.P1[:], c.RU[:, 1, :], c.zsl,
                                            OP.mult)
                    c.OMU = gp.tile([P, cw], BF16, tag="E", name="OMU")
                    omu_eng = nc.gpsimd if OMU_POOL else nc.vector
                    omu_eng.tensor_scalar(c.OMU[:], c.RU[:, 1, :], -1.0,
                                          1.0, OP.mult, OP.add)
                return c

            def stage_n(c):
                NT = gp.tile([P, c.cw], BF16, tag="NT", name="NT")
                if c.t0c:
                    # i_n = b_ihn at t=0: tanh reads T1 straight from SBUF
                    nc.scalar.activation(NT[:], c.T1[:], AF.Tanh,
                                         bias=bins[:, 0:1])
                else:
                    # reuse the bank: fresh group = lni@wp + T1 via identity
                    nc.tensor.matmul(c.pB[:], lnis[:], c.wprev,
                                     start=True, stop=False,
                                     skip_group_check=True)
                    nc.tensor.matmul(c.pB[:], eyes[:], c.T1[:],
                                     start=False, stop=True,
                                     skip_group_check=True)
                    nc.scalar.activation(NT[:], c.pB[:], AF.Tanh)
                if c.fast:
                    P2 = gp.tile([P, c.cw], BF16, tag="P2", name="P2")
                    nc.vector.tensor_tensor(P2[:], c.OMU[:], NT[:], OP.mult)
                    nc.vector.tensor_tensor(c.znew, c.P1[:], P2[:], OP.add)
                else:
                    D = gp.tile([P, c.cw], BF16, tag="D", name="D")
                    nc.gpsimd.tensor_tensor(D[:], c.zsl, NT[:], OP.subtract)
                    E = gp.tile([P, c.cw], BF16, tag="E", name="E")
                    if c.par == 0:
                        nc.gpsimd.tensor_tensor(E[:], c.RU[:, 1, :], D[:],
                                                OP.mult)
                    else:
                        nc.vector.tensor_tensor(E[:], c.RU[:, 1, :], D[:],
                                                OP.mult)
                    zq = nc.gpsimd if ZN_POOL else nc.vector
                    zq.tensor_tensor(c.znew, E[:], NT[:], OP.add)

            def stage_w(c):
                pacc = paccA if c.m < G else paccB
                off = 32 * (c.m % G)
                nc.tensor.matmul(pacc[:, c.c0:c.c0 + c.cw], lws[:, c.m, :],
                                 c.znew,
                                 start=(c.t == 0 and c.m % G == 0),
                                 stop=(c.t == nsteps - 1 and
                                       c.m % G == G - 1 and
                                       c.c0 + c.cw == BT),
                                 skip_group_check=True)
                if DRAIN_ACT or (c.fast and c.par == 0):
                    nc.scalar.activation(
                        wpball[0:2 * G, c.t + 1, c.m, c.c0:c.c0 + c.cw],
                        pacc[off:off + 2 * G, c.c0:c.c0 + c.cw],
                        AF.Identity,
                        bias=bwts[off:off + 2 * G, c.t:c.t + 1])
                else:
                    nc.vector.tensor_scalar_add(
                        wpball[0:2 * G, c.t + 1, c.m, c.c0:c.c0 + c.cw],
                        pacc[off:off + 2 * G, c.c0:c.c0 + c.cw],
                        bwts[off:off + 2 * G, c.t:c.t + 1])
                if T9_STORE and c.t == nsteps - 1 and c.c0 + c.cw == BT:
                    # last step: store this macro immediately so the final
                    # barrier doesn't wait on one big trailing DMA
                    nc.sync.dma_start(outd[:, c.t, c.m, :],
                                      wpball[0:2 * G, c.t + 1, c.m, :])

            # pipeline state
            next_t = [0] * NM
            ndone = [-1] * NM      # highest t with stage_n emitted
            wdone = [-1] * NM      # highest t with stage_w emitted
            pend_n = []            # gates emitted, stage_n pending
            pend_w = []            # stage_n emitted, stage_w pending
            dma_t = [0]

            TCAP = nsteps - 1 if T9_STORE else nsteps

            def flush_dmas(slack):
                while (dma_t[0] < TCAP
                       and all(w >= dma_t[0] + slack for w in wdone)):
                    t0 = dma_t[0]
                    nc.sync.dma_start(outd[:, t0, :, :],
                                      wpball[0:2 * G, t0 + 1, :, :])
                    dma_t[0] = t0 + 1

            def batch(mm_max, dcap, fast=False):
                nsnap = list(ndone)
                wsnap = list(wdone)
                for c in pend_n:
                    stage_n(c)
                    ndone[c.m] = c.t
                moved = pend_n[:]
                pend_n.clear()
                old_w = pend_w[:]
                pend_w.clear()
                specs = []
                for m in range(mm_max):
                    t = next_t[m]
                    if (t < nsteps and t + m <= dcap
                            and len(specs) < int(_os.environ.get("K_CAP", "4"))
                            and (t == 0 or (nsnap[m] >= t - 1
                                            and wsnap[m] >= t - 1))):
                        specs.append((t, m))
                        next_t[m] = t + 1
                for t, m in specs:
                    pend_n.append(stage_gates(t, m, fast=fast))
                pend_w.extend(moved)

                def emit_w():
                    for c in old_w:
                        stage_w(c)
                        wdone[c.m] = c.t
                    flush_dmas(1)
                if WMID:
                    return emit_w
                emit_w()
                return None

            DC = int(_os.environ.get("K_DC", "2"))
            for s in range(NM):
                cb = batch(s, 2 * s - DC)
                mlp_pair(G * s, first=(s == 0), mid_cb=cb)
                if s == 0:
                    load_gru_consts()
                cb = batch(s, 2 * s - DC + 1)
                mlp_pair(G * s + 2, mid_cb=cb)
            # drain weave: greedy most-starved chain, software-pipelined,
            # with chain cadence >= 3 slots enforced via the stage snapshots
            SP_W = int(_os.environ.get("K_SPW", "6"))
            lastg = [-SP_W] * NM
            pos = 0
            while (any(t < nsteps for t in next_t) or pend_n or pend_w):
                nsnap = list(ndone)
                wsnap = list(wdone)
                for c in pend_n:
                    stage_n(c)
                    ndone[c.m] = c.t
                moved = pend_n[:]
                pend_n.clear()

                def elig(m, snapn, snapw):
                    t = next_t[m]
                    return (t < nsteps
                            and (t == 0 or (snapn[m] >= t - 1
                                            and snapw[m] >= t - 1)))

                cand = [m for m in range(NM)
                        if elig(m, nsnap, wsnap) and pos - lastg[m] >= SP_W]
                if not cand:
                    cand = [m for m in range(NM) if elig(m, nsnap, wsnap)]
                if not cand and not moved and not pend_w:
                    cand = [m for m in range(NM)
                            if elig(m, ndone, wdone)]
                if cand:
                    m = max(cand, key=lambda mm: nsteps - next_t[mm])
                    t = next_t[m]
                    next_t[m] = t + 1
                    pend_n.append(stage_gates(t, m, fast=True))
                    lastg[m] = pos
                for c in pend_w:
                    stage_w(c)
                    wdone[c.m] = c.t
                pend_w.clear()
                pend_w.extend(moved)
                pos += 1
                flush_dmas(1)
            while dma_t[0] < TCAP:
                t0 = dma_t[0]
                nc.sync.dma_start(outd[:, t0, :, :],
                                  wpball[0:2 * G, t0 + 1, :, :])
                dma_t[0] = t0 + 1
    nc.compile()
    return nc


LAST_RESULT = None


def _lwd(Ww):
    I4 = np.eye(G, dtype=np.float32)
    blk = np.kron(I4, Ww.T)                      # [128, 8]
    out = np.zeros((P, NM, P), np.float32)
    for m in range(NM):
        off = 32 * (m % G)
        out[:, m, off:off + 2 * G] = blk
    return out.astype(np_bf16)


def _zod(nsteps):
    # ones-row only: wp rows are always drain-written before read
    return np.ones((1, nsteps + 1, NM, BT), np.float32).astype(np_bf16)


def _prep_common(inputs, nsteps):
    W1 = np.asarray(inputs["W1"], np.float32)
    b1 = np.asarray(inputs["b1"], np.float32)
    W2 = np.asarray(inputs["W2"], np.float32)
    b2 = np.asarray(inputs["b2"], np.float32)
    Wm = np.asarray(inputs["Wm"], np.float32)
    bm = np.asarray(inputs["bm"], np.float32)
    w_ih = np.asarray(inputs["w_ih"], np.float32)
    w_hh = np.asarray(inputs["w_hh"], np.float32)
    b_ih = np.asarray(inputs["b_ih"], np.float32)
    b_hh = np.asarray(inputs["b_hh"], np.float32)
    Ww = np.asarray(inputs["Ww"], np.float32)
    bw = np.asarray(inputs["bw"], np.float32)

    I4 = np.eye(G, dtype=np.float32)

    def aug(gate_w, bias_row):
        m = np.zeros((2 * G + 1, P), np.float32)
        m[0:2 * G] = np.kron(I4, gate_w.T)
        m[2 * G] = np.tile(bias_row, G)
        return m.astype(np_bf16)

    bwt8 = np.outer(np.tile(bw, G),
                    np.arange(1, nsteps + 1)).astype(np.float32)
    bwt = np.zeros((P, nsteps), np.float32)
    for k in range(G):
        bwt[32 * k:32 * k + 2 * G] = bwt8
    common = {
        "w1d": np.ascontiguousarray(
            W1.T.reshape(2, P, H).transpose(1, 0, 2)).astype(np_bf16),
        "w2d": np.ascontiguousarray(
            W2.T.reshape(4, P, H).transpose(1, 0, 2)).astype(np_bf16),
        "wmd": np.ascontiguousarray(
            Wm.T.reshape(4, P, A).transpose(1, 0, 2)).astype(np_bf16),
        "b1d": np.ascontiguousarray(b1.reshape(4, P).T),
        "b2d": np.ascontiguousarray(b2.reshape(4, P).T),
        "bmd": bm.reshape(A, 1).copy(),
        "lrid": aug(w_ih[0:A], b_ih[0:A] + b_hh[0:A]),
        "luid": aug(w_ih[A:2 * A], b_ih[A:2 * A] + b_hh[A:2 * A]),
        "lnid": aug(w_ih[2 * A:3 * A], b_ih[2 * A:3 * A]),
        "lrhd": np.ascontiguousarray(np.kron(I4, w_hh[0:A].T)).astype(np_bf16),
        "luhd": np.ascontiguousarray(
            np.kron(I4, w_hh[A:2 * A].T)).astype(np_bf16),
        "lnhd": np.ascontiguousarray(
            np.kron(I4, w_hh[2 * A:3 * A].T)).astype(np_bf16),
        "lwd": _lwd(Ww),
        "zod": _zod(nsteps),
        "eyed": np.eye(P, dtype=np.float32).astype(np_bf16),
        "bnhd": np.tile(b_hh[2 * A:3 * A], G).reshape(P, 1).copy(),
        "bwtd": bwt,
        "brd": np.tile(b_ih[0:A] + b_hh[0:A], G).reshape(P, 1).copy(),
        "bud": np.tile(b_ih[A:2 * A] + b_hh[A:2 * A], G).reshape(P, 1).copy(),
        "bind": np.tile(b_ih[2 * A:3 * A], G).reshape(P, 1).copy(),
    }
    return common


def kernel(**inputs) -> np.ndarray:
    global LAST_RESULT
    x = np.asarray(inputs["x"], dtype=np.float32)
    T = int(inputs["pred_length"])

    common = _prep_common(inputs, T)
    # x -> [P, 2, BC] per core: xd[p, kb, n] = x[n, kb*128+p]
    xT = np.ascontiguousarray(x.T.astype(np_bf16))      # [S, B]
    xT = xT.reshape(2, P, B)
    in_maps = []
    for i in range(NCORES):
        m = dict(common)
        m["xd"] = np.ascontiguousarray(
            xT[:, :, i * BC:(i + 1) * BC].transpose(1, 0, 2))
        in_maps.append(m)

    if T not in _CACHE:
        _CACHE[T] = _build(T)
    nc = _CACHE[T]
    res = run_bass_kernel_spmd(nc, in_maps, core_ids=list(range(NCORES)))
    LAST_RESULT = res
    parts = []
    for i in range(NCORES):
        o = np.asarray(res.results[i]["outd"]).astype(np.float32)
        # o[2g+j, t, m, n] -> out[m*2048 + g*512 + n, 2t+j]
        o = o.reshape(G, 2, T, NM, BT).transpose(3, 0, 4, 2, 1)
        parts.append(o.reshape(BC, 2 * T))
    return np.ascontiguousarray(np.concatenate(parts, axis=0))

